# revision 39
# baseline (speedup 1.0000x reference)
"""MixLoRA sparse-MoE Trainium2 kernel (8 NeuronCores, token-sharded SPMD).

Math (per token t):
  logits = h @ gate_w.T ; rw = softmax(logits); top-2 renormalized -> w_te [T,E]
  cg = h @ Wg.T ; cu = h @ Wu.T                       (shared base projections)
  g_e = cg + 2*(h @ Ag_e.T) @ Bg_e.T
  u_e = cu + 2*(h @ Au_e.T) @ Bu_e.T
  act_e = silu(g_e) * u_e
  mix   = sum_e w_te[:,e] * act_e                      ("mix trick": one down proj)
  out   = mix @ Wd.T + sum_e 2*((w_te[:,e]*act_e) @ Ad_e.T) @ Bd_e.T

Sharding: tokens split 8 ways (256/core); all weights replicated (they are
shared across experts; only rank-16 LoRA factors are per-expert, so
expert-parallel all-to-all would lose).  Each core returns outT [H, 256];
host concatenates and transposes.  No collectives.

Layout on core: F-on-partitions.  All big tensors are pre-transposed on the
host so no on-chip transposes of activations are ever needed.
"""

import os
import numpy as np
import ml_dtypes
from contextlib import ExitStack

import concourse.bass as bass
import concourse.mybir as mybir
import concourse.tile as tile
from concourse import bacc
from concourse.bass_utils import run_bass_kernel_spmd

BF16 = mybir.dt.bfloat16
F32 = mybir.dt.float32
AF = mybir.ActivationFunctionType
OP = mybir.AluOpType
AX = mybir.AxisListType

T, H, F, E, R = 2048, 2048, 8192, 8, 16
NCORES = 8
TC = T // NCORES            # 256 tokens per core
HK = H // 128               # 16 H-chunks
FK = F // 128               # 64 F-chunks
LORA = 2.0

_nb = ml_dtypes.bfloat16

LAST_EXEC_NS = None
LAST_TRACE = None
_NC_CACHE = {}
SILU_MODE = "silu"  # "silu" (HW ACT table) | "sigmoid_mul" (CoreSim-compatible)


def _bf(x):
    return np.ascontiguousarray(x.astype(_nb))


def _hi_lo(x):
    hi = x.astype(_nb)
    lo = (x - hi.astype(np.float32)).astype(_nb)
    return np.ascontiguousarray(hi), np.ascontiguousarray(lo)


def _build_nc():
    PH = int(os.environ.get("KERNEL_PHASES", "3"))
    nc = bacc.Bacc()

    # ---- per-core DRAM parameters ----------------------------------------
    hT_hi_d = nc.declare_dram_parameter("hT_hi", [H, TC], BF16, isOutput=False)
    hT_lo_d = nc.declare_dram_parameter("hT_lo", [H, TC], BF16, isOutput=False)
    gw_hi_d = nc.declare_dram_parameter("gw_hi", [H, E], BF16, isOutput=False)
    gw_lo_d = nc.declare_dram_parameter("gw_lo", [H, E], BF16, isOutput=False)
    WgT_d = nc.declare_dram_parameter("WgT", [H, F], BF16, isOutput=False)
    WuT_d = nc.declare_dram_parameter("WuT", [H, F], BF16, isOutput=False)
    WdT_d = nc.declare_dram_parameter("WdT", [F, H], BF16, isOutput=False)
    # A-pack [H, 4*128]: j-blocks (gA, gB, uA, uB); within block, expert strip
    # i occupies cols 32i..32i+16 holding A[e].T (e = i or i+4).
    Apack_d = nc.declare_dram_parameter("Apack", [H, 512], BF16, isOutput=False)
    # B-pack [128, 4, F]: j-blocks (gA, gB, uA, uB); rows 32i..32i+16 of block
    # j hold 2*B[e].T  ([R, F]).
    Bpack_d = nc.declare_dram_parameter("Bpack", [128, 4, F], BF16, isOutput=False)
    # Ad-pack [F, 2*128]: blocks (A, B); cols 32i..32i+16 hold Ad[e].T ([F, R])
    Adpack_d = nc.declare_dram_parameter("Adpack", [F, 256], BF16, isOutput=False)
    # Bd-pack [128, 2, H]: blocks (A, B); rows 32i..32i+16 hold 2*Bd[e].T
    Bdpack_d = nc.declare_dram_parameter("Bdpack", [128, 2, H], BF16, isOutput=False)

    outT_d = nc.declare_dram_parameter("outT", [H, TC], F32, isOutput=True)
    w_scratch = nc.dram_tensor("w_scratch", [TC, E], BF16)

    with ExitStack() as ctx:
        tc = ctx.enter_context(tile.TileContext(nc))

        # ---- resident SBUF ------------------------------------------------
        res = ctx.enter_context(tc.tile_pool(name="res", bufs=1))
        hT_hi = res.tile([128, HK * TC], BF16)          # 8 KB/p
        cguT = res.tile([128, FK * 512], BF16)          # 64 KB/p
        mixT = res.tile([128, FK * TC], BF16)           # 32 KB/p
        wb_sb = res.tile([128, E * TC], BF16)           # 4 KB/p
        z_sb = res.tile([128, 4 * TC], BF16)            # 2 KB/p
        zdT_sb = res.tile([128, 2 * TC], BF16)          # 1 KB/p
        bd_sb = res.tile([128, 2 * H], BF16)            # 8 KB/p
        ones_bf = res.tile([1, 128], BF16)
        # zdT rows 32i+16..32i+32 are never written but read by the dense
        # K=128 Bd matmul (against zero weights) -> must be finite
        nc.vector.memset(zdT_sb, 0.0)

        nc.sync.dma_start(
            hT_hi.rearrange("p (hk t) -> p hk t", hk=HK),
            hT_hi_d.ap().rearrange("(hk p) t -> p hk t", p=128))
        nc.sync.dma_start(
            bd_sb.rearrange("p (j h) -> p j h", j=2), Bdpack_d.ap())
        nc.vector.memset(ones_bf, 1.0)

        # ---- phase 0: router + z + broadcasts -----------------------------
        p0ctx = ExitStack()
        p0 = p0ctx.enter_context(tc.tile_pool(name="p0", bufs=1))
        hT_lo = p0.tile([128, HK * TC], BF16)
        Apk = p0.tile([128, HK * 512], BF16)
        gw_hi = p0.tile([128, HK * E], BF16)
        gw_lo = p0.tile([128, HK * E], BF16)
        w_row = p0.tile([1, E * TC], BF16)

        nc.sync.dma_start(
            hT_lo.rearrange("p (hk t) -> p hk t", hk=HK),
            hT_lo_d.ap().rearrange("(hk p) t -> p hk t", p=128))
        nc.sync.dma_start(
            Apk.rearrange("p (hk a) -> p hk a", hk=HK),
            Apack_d.ap().rearrange("(hk p) a -> p hk a", p=128))
        nc.sync.dma_start(
            gw_hi.rearrange("p (hk e) -> p hk e", hk=HK),
            gw_hi_d.ap().rearrange("(hk p) e -> p hk e", p=128))
        nc.sync.dma_start(
            gw_lo.rearrange("p (hk e) -> p hk e", hk=HK),
            gw_lo_d.ap().rearrange("(hk p) e -> p hk e", p=128))

        pr = p0ctx.enter_context(tc.tile_pool(name="pr", bufs=2, space="PSUM"))
        sr = p0ctx.enter_context(tc.tile_pool(name="sr", bufs=2))

        ntt = TC // 128  # token tiles per core (2)
        wte_bf_all = p0.tile([128, ntt * E], BF16)
        for tt in range(ntt):
            lg_ps = pr.tile([128, E], F32, tag="r")
            passes = [(hT_hi, gw_hi), (hT_hi, gw_lo), (hT_lo, gw_hi)]
            n = 0
            for (hsrc, gsrc) in passes:
                for hk in range(HK):
                    nc.tensor.matmul(
                        lg_ps,
                        lhsT=hsrc[:, hk * TC + tt * 128:hk * TC + tt * 128 + 128],
                        rhs=gsrc[:, hk * E:(hk + 1) * E],
                        start=(n == 0), stop=(n == 3 * HK - 1))
                    n += 1
            lg = sr.tile([128, E], F32, tag="lg")
            nc.vector.tensor_copy(lg, lg_ps)
            m1 = sr.tile([128, 1], F32, tag="m1")
            nc.vector.tensor_reduce(m1, lg, AX.X, OP.max)
            nm1 = sr.tile([128, 1], F32, tag="nm1")
            nc.vector.tensor_scalar_mul(nm1, m1, -1.0)
            ex = sr.tile([128, E], F32, tag="ex")
            nc.scalar.activation(ex, lg, AF.Exp, bias=nm1[:, 0:1])
            sm = sr.tile([128, 1], F32, tag="sm")
            nc.vector.tensor_reduce(sm, ex, AX.X, OP.add)
            sinv = sr.tile([128, 1], F32, tag="sinv")
            nc.vector.reciprocal(sinv, sm)
            rw = sr.tile([128, E], F32, tag="rw")
            nc.vector.tensor_scalar_mul(rw, ex, sinv[:, 0:1])
            r1 = sr.tile([128, 1], F32, tag="r1")
            nc.vector.tensor_reduce(r1, rw, AX.X, OP.max)
            mask1 = sr.tile([128, E], F32, tag="mask1")
            nc.vector.tensor_scalar(mask1, rw, r1[:, 0:1], None, OP.is_ge)
            rw2 = sr.tile([128, E], F32, tag="rw2")
            nc.vector.scalar_tensor_tensor(rw2, mask1, -2.0, rw, OP.mult, OP.add)
            r2 = sr.tile([128, 1], F32, tag="r2")
            nc.vector.tensor_reduce(r2, rw2, AX.X, OP.max)
            wsum = sr.tile([128, 1], F32, tag="wsum")
            nc.vector.tensor_add(wsum, r1, r2)
            winv = sr.tile([128, 1], F32, tag="winv")
            nc.vector.reciprocal(winv, wsum)
            selm = sr.tile([128, E], F32, tag="selm")
            nc.vector.tensor_scalar(selm, rw, r2[:, 0:1], None, OP.is_ge)
            wte = sr.tile([128, E], F32, tag="wte")
            nc.vector.scalar_tensor_tensor(
                wte, rw, winv[:, 0:1], selm, OP.mult, OP.mult)
            nc.vector.tensor_copy(
                wte_bf_all[:, tt * E:(tt + 1) * E], wte)
        nc.sync.dma_start(
            w_scratch.ap().rearrange("(tt p) e -> p tt e", p=128),
            wte_bf_all.rearrange("p (tt e) -> p tt e", tt=ntt))
        # gather back transposed: w_row[0, e*TC + t] = w_scratch[t, e]
        nc.sync.dma_start(
            w_row.rearrange("o (e t) -> o e t", e=E),
            w_scratch.ap().rearrange("t e -> e t"))
        for e in range(E):
            wb_ps = pr.tile([128, TC], F32, tag="r")
            nc.tensor.matmul(
                wb_ps, lhsT=ones_bf, rhs=w_row[0:1, e * TC:(e + 1) * TC],
                start=True, stop=True)
            nc.scalar.copy(wb_sb[:, e * TC:(e + 1) * TC], wb_ps)

        # z = A-projections of h: 4 pack groups (gA, gB, uA, uB)
        for j in range(4):
            z_ps = pr.tile([128, TC], F32, tag="r")
            for hk in range(HK):
                nc.tensor.matmul(
                    z_ps,
                    lhsT=Apk[:, hk * 512 + j * 128:hk * 512 + (j + 1) * 128],
                    rhs=hT_hi[:, hk * TC:(hk + 1) * TC],
                    start=(hk == 0), stop=(hk == HK - 1))
            nc.scalar.copy(z_sb[:, j * TC:(j + 1) * TC], z_ps)

        p0ctx.close()

        # ---- phase 1: cg / cu ---------------------------------------------
        p1ctx = ExitStack()
        wpool = p1ctx.enter_context(tc.tile_pool(name="wp", bufs=3))
        pgu_pool = p1ctx.enter_context(tc.tile_pool(name="pgu", bufs=2, space="PSUM"))
        for fk in range(FK if PH >= 1 else 0):
            wg_t = wpool.tile([128, HK * 128], BF16, tag="wg")
            wu_t = wpool.tile([128, HK * 128], BF16, tag="wu")
            nc.sync.dma_start(
                wg_t.rearrange("p (hk f) -> p hk f", hk=HK),
                WgT_d.ap()[:, fk * 128:(fk + 1) * 128]
                .rearrange("(hk p) f -> p hk f", p=128))
            nc.sync.dma_start(
                wu_t.rearrange("p (hk f) -> p hk f", hk=HK),
                WuT_d.ap()[:, fk * 128:(fk + 1) * 128]
                .rearrange("(hk p) f -> p hk f", p=128))
            pgu = pgu_pool.tile([128, 512], F32, tag="gu")
            for hk in range(HK):
                nc.tensor.matmul(
                    pgu[:, 0:TC],
                    lhsT=wg_t[:, hk * 128:(hk + 1) * 128],
                    rhs=hT_hi[:, hk * TC:(hk + 1) * TC],
                    start=(hk == 0), stop=(hk == HK - 1))
            for hk in range(HK):
                nc.tensor.matmul(
                    pgu[:, TC:512],
                    lhsT=wu_t[:, hk * 128:(hk + 1) * 128],
                    rhs=hT_hi[:, hk * TC:(hk + 1) * TC],
                    start=(hk == 0), stop=(hk == HK - 1))
            nc.scalar.copy(cguT[:, fk * 512:fk * 512 + TC], pgu[:, 0:TC])
            nc.vector.tensor_copy(
                cguT[:, fk * 512 + TC:(fk + 1) * 512], pgu[:, TC:512])

        p1ctx.close()

        # ---- phase 2: per-expert LoRA deltas + silu + mix -----------------
        p2ctx = ExitStack()
        bpool = p2ctx.enter_context(tc.tile_pool(name="bp", bufs=3))
        dpool = p2ctx.enter_context(tc.tile_pool(name="dp", bufs=1, space="PSUM"))
        zdpool = p2ctx.enter_context(tc.tile_pool(name="zdp", bufs=1, space="PSUM"))
        epool = p2ctx.enter_context(tc.tile_pool(name="ep", bufs=4))

        zd_ps = [zdpool.tile([128, TC], F32, tag=f"zd{h}", name=f"zd_ps{h}")
                 for h in range(2)]

        for fk in range(FK if PH >= 2 else 0):
            b_t = bpool.tile([128, 4 * 128], BF16, tag="b")
            nc.sync.dma_start(
                b_t.rearrange("p (j f) -> p j f", j=4),
                Bpack_d.ap()[:, :, fk * 128:(fk + 1) * 128])
            ad_t = bpool.tile([128, 256], BF16, tag="ad")
            nc.sync.dma_start(ad_t, Adpack_d.ap()[fk * 128:(fk + 1) * 128, :])

            for half in range(2):
                zg = z_sb[:, half * TC:(half + 1) * TC]
                zu = z_sb[:, (2 + half) * TC:(3 + half) * TC]
                p_es = []
                for i in range(4):
                    p_e = dpool.tile([128, 512], F32, tag=f"delta{i}",
                                     name=f"p_e{i}")
                    nc.tensor.matmul(
                        p_e[:, 0:TC],
                        lhsT=b_t[32 * i:32 * i + 16,
                                 half * 128:(half + 1) * 128],
                        rhs=zg[32 * i:32 * i + 16, :],
                        start=True, stop=True, tile_position=(32 * i, 0))
                    nc.tensor.matmul(
                        p_e[:, TC:512],
                        lhsT=b_t[32 * i:32 * i + 16,
                                 (2 + half) * 128:(3 + half) * 128],
                        rhs=zu[32 * i:32 * i + 16, :],
                        start=True, stop=True, tile_position=(32 * i, 0))
                    p_es.append(p_e)
                for i in range(4):
                    e = 4 * half + i
                    p_e = p_es[i]
                    gu = epool.tile([128, 512], BF16, tag="gu")
                    nc.scalar.copy(gu, p_e)
                    guf = epool.tile([128, 512], BF16, tag="guf")
                    nc.vector.tensor_add(
                        guf, gu, cguT[:, fk * 512:(fk + 1) * 512])
                    s = epool.tile([128, TC], BF16, tag="s")
                    p = epool.tile([128, TC], BF16, tag="p")
                    if SILU_MODE == "silu":
                        nc.scalar.activation(s, guf[:, 0:TC], AF.Silu)
                        nc.vector.tensor_mul(p, s, guf[:, TC:512])
                    else:
                        nc.scalar.activation(s, guf[:, 0:TC], AF.Sigmoid)
                        p1 = epool.tile([128, TC], BF16, tag="p1")
                        nc.vector.tensor_mul(p1, s, guf[:, TC:512])
                        nc.vector.tensor_mul(p, p1, guf[:, 0:TC])
                    pw = epool.tile([128, TC], BF16, tag="pw")
                    nc.vector.tensor_mul(pw, p, wb_sb[:, e * TC:(e + 1) * TC])
                    if e == 0:
                        nc.vector.tensor_copy(
                            mixT[:, fk * TC:(fk + 1) * TC], pw)
                    else:
                        nc.vector.tensor_add(
                            mixT[:, fk * TC:(fk + 1) * TC],
                            mixT[:, fk * TC:(fk + 1) * TC], pw)
                    nc.tensor.matmul(
                        zd_ps[half][32 * i:32 * i + 16, :],
                        lhsT=ad_t[:, half * 128 + 32 * i:
                                  half * 128 + 32 * i + 16],
                        rhs=pw,
                        start=(fk == 0), stop=(fk == FK - 1),
                        tile_position=(0, 32 * i), skip_group_check=True)

        if PH >= 2:
            for half in range(2):
                for i in range(4):
                    nc.scalar.copy(
                        zdT_sb[32 * i:32 * i + 16, half * TC:(half + 1) * TC],
                        zd_ps[half][32 * i:32 * i + 16, :])
        else:
            nc.vector.memset(zdT_sb, 0.0)
            nc.vector.memset(mixT, 0.0)

        p2ctx.close()

        # ---- phase 3: down projection -------------------------------------
        wdpool = ctx.enter_context(tc.tile_pool(name="wdp", bufs=2))
        opool = ctx.enter_context(tc.tile_pool(name="op", bufs=1, space="PSUM"))
        ospool = ctx.enter_context(tc.tile_pool(name="osp", bufs=3))
        if PH < 3:
            zt = ospool.tile([128, TC], F32, tag="osb")
            nc.vector.memset(zt, 0.0)
            for hk in range(HK):
                nc.sync.dma_start(outT_d.ap()[hk * 128:(hk + 1) * 128, :], zt)
        for hk in range(HK if PH >= 3 else 0):
            wd_t = wdpool.tile([128, FK * 128], BF16, tag="wd")
            nc.sync.dma_start(
                wd_t.rearrange("p (fk hh) -> p fk hh", fk=FK),
                WdT_d.ap()[:, hk * 128:(hk + 1) * 128]
                .rearrange("(fk p) hh -> p fk hh", p=128))
            po = opool.tile([128, TC], F32, tag="o")
            for fk in range(FK):
                nc.tensor.matmul(
                    po, lhsT=wd_t[:, fk * 128:(fk + 1) * 128],
                    rhs=mixT[:, fk * TC:(fk + 1) * TC],
                    start=(fk == 0), stop=False, skip_group_check=True)
            # Bd LoRA term: zero rows in the strip-packed stationary kill the
            # unused zdT rows, so this is a plain full-K matmul per half.
            for half in range(2):
                nc.tensor.matmul(
                    po,
                    lhsT=bd_sb[:, half * H + hk * 128:half * H + (hk + 1) * 128],
                    rhs=zdT_sb[:, half * TC:(half + 1) * TC],
                    start=False, stop=(half == 1), skip_group_check=True)
            o_sb = ospool.tile([128, TC], F32, tag="osb")
            nc.scalar.copy(o_sb, po)
            nc.sync.dma_start(outT_d.ap()[hk * 128:(hk + 1) * 128, :], o_sb)

    nc.compile()
    return nc


def _prep_inputs(hidden_states, gate_w, Wg, Wu, Wd, Ag, Bg, Au, Bu, Ad, Bd):
    h = np.asarray(hidden_states, np.float32).reshape(T, H)
    hT = np.ascontiguousarray(h.T)                     # [H, T]
    hT_hi, hT_lo = _hi_lo(hT)
    gwT = np.ascontiguousarray(np.asarray(gate_w, np.float32).T)  # [H, E]
    gw_hi, gw_lo = _hi_lo(gwT)

    WgT = _bf(np.asarray(Wg, np.float32).T)            # [H, F]
    WuT = _bf(np.asarray(Wu, np.float32).T)
    WdT = _bf(np.asarray(Wd, np.float32).T)            # [F, H]

    Ag = np.asarray(Ag, np.float32)                    # [E, R, H]
    Au = np.asarray(Au, np.float32)
    Bg = np.asarray(Bg, np.float32)                    # [E, F, R]
    Bu = np.asarray(Bu, np.float32)
    Ad = np.asarray(Ad, np.float32)                    # [E, R, F]
    Bd = np.asarray(Bd, np.float32)                    # [E, H, R]

    Apack = np.zeros((H, 512), np.float32)
    for j, A in enumerate((Ag[:4], Ag[4:], Au[:4], Au[4:])):
        for i in range(4):
            Apack[:, j * 128 + 32 * i:j * 128 + 32 * i + R] = A[i].T
    Bpack = np.zeros((128, 4, F), np.float32)
    for j, B in enumerate((Bg[:4], Bg[4:], Bu[:4], Bu[4:])):
        for i in range(4):
            Bpack[32 * i:32 * i + R, j, :] = LORA * B[i].T
    Adpack = np.zeros((F, 256), np.float32)
    for j, A in enumerate((Ad[:4], Ad[4:])):
        for i in range(4):
            Adpack[:, j * 128 + 32 * i:j * 128 + 32 * i + R] = A[i].T
    Bdpack = np.zeros((128, 2, H), np.float32)
    for j, B in enumerate((Bd[:4], Bd[4:])):
        for i in range(4):
            Bdpack[32 * i:32 * i + R, j, :] = LORA * B[i].T

    shared = {
        "gw_hi": gw_hi, "gw_lo": gw_lo,
        "WgT": WgT, "WuT": WuT, "WdT": WdT,
        "Apack": _bf(Apack), "Bpack": _bf(Bpack),
        "Adpack": _bf(Adpack), "Bdpack": _bf(Bdpack),
    }
    in_maps = []
    for c in range(NCORES):
        m = dict(shared)
        m["hT_hi"] = np.ascontiguousarray(hT_hi[:, c * TC:(c + 1) * TC])
        m["hT_lo"] = np.ascontiguousarray(hT_lo[:, c * TC:(c + 1) * TC])
        in_maps.append(m)
    return in_maps


def _ensure_ntff_hook():
    """Register the axon NTFF-profile hook if this image's antenv lacks
    axon_hooks (degrades silently; tracing is optional)."""
    import sys, types
    try:
        import antenv.axon_hooks  # noqa: F401
        return
    except ImportError:
        pass
    try:
        import antenv
        mod = types.ModuleType("antenv.axon_hooks")
        _h = [None]
        mod.set_axon_ntff_profile_hook = lambda h: _h.__setitem__(0, h)
        mod.get_axon_ntff_profile_hook = lambda: _h[0]
        sys.modules["antenv.axon_hooks"] = mod
        antenv.axon_hooks = mod
        from trn_agent_boot.trn_boot import _ntff_profile_via_ctypes
        hook = _ntff_profile_via_ctypes("/opt/axon/libaxon_pjrt.so")
        mod.set_axon_ntff_profile_hook(hook)
    except Exception:
        pass


def kernel(hidden_states, gate_w, Wg, Wu, Wd, Ag, Bg, Au, Bu, Ad, Bd, top_k):
    global LAST_EXEC_NS, LAST_TRACE
    assert int(top_k) == 2
    os.environ.setdefault("MYCRO_LOCAL_CACHE", "1")
    _ensure_ntff_hook()

    if "nc" not in _NC_CACHE:
        _NC_CACHE["nc"] = _build_nc()
    nc = _NC_CACHE["nc"]

    in_maps = _prep_inputs(hidden_states, gate_w, Wg, Wu, Wd,
                           Ag, Bg, Au, Bu, Ad, Bd)
    trace = bool(os.environ.get("BASS_KERNEL_TRACE"))
    ncores = int(os.environ.get("KERNEL_CORES", str(NCORES)))
    res = run_bass_kernel_spmd(nc, in_maps[:ncores], core_ids=list(range(ncores)),
                               trace=trace)
    LAST_EXEC_NS = res.exec_time_ns
    if res.instructions_and_trace is not None:
        LAST_TRACE = res.instructions_and_trace[1]

    outT = np.concatenate([np.asarray(r["outT"]) for r in res.results], axis=1)
    if outT.shape[1] < T:  # debug path (KERNEL_CORES < 8)
        outT = np.pad(outT, ((0, 0), (0, T - outT.shape[1])))
    return np.ascontiguousarray(outT.T, np.float32).reshape(1, T, H)
